# revision 12
# baseline (speedup 1.0000x reference)
"""Multi-head attention (B=2, S=2048, D=1024, H=16) on 8 NeuronCores.

Sharding: core c -> (batch b = c // 4, head-group g = c % 4, 4 heads each).
Each core computes its 4 heads' attention for its batch plus the partial
output projection (ctx_shard @ WO_shard.T).T; the host sums the 4 partials
per batch, adds the bias, and patches fully-masked query rows (where the
reference's softmax degenerates to uniform attention).

Device kernel layout notes:
  - x and the weight shards are pre-transposed on the host and fed as bf16.
  - Q,K are produced in [dk, s] layout (head-pair stacked on partitions) so
    scores come out transposed: S_t[k, q]. The two heads of a pair run as
    concurrent row-group matmuls (K=64 each).
  - Padding mask is folded into the exp as a per-partition (=per-key)
    activation bias of -30 on masked keys: exp(s/8 - 30) ~ 0, so P rows of
    masked keys vanish and V needs no masking at all. Causal mask is a
    0/1-triangle multiply on the band tiles (DVE).
  - Softmax normalization is deferred: V carries an extra ones column so
    P@V also accumulates row sums L[q]; unnormalized ctx and L stage to
    SBUF, then reciprocal_approx_fast + DRAM-bounce partition-broadcast
    divides at the end (pair 0 pipelined under pair-1 attention; pair 1
    per-qt).
  - Emission is a single software pipeline: x streams in 512-column
    stripes; Q/K/V projections and the output projection are emitted as
    small "filler" units inside the exp-paced attention loops so the PE
    never drains; y tiles DMA out as soon as each out-projection column
    block finishes.
"""

import os
import sys

import numpy as np

sys.path.insert(0, "/opt/trn_rl_repo")
os.environ.setdefault("MYCRO_LOCAL_CACHE", "1")

import ml_dtypes

import concourse.bass as bass
import concourse.tile as tile
from concourse import bacc, mybir
from concourse.bass_utils import run_bass_kernel_spmd

B, S, D, H = 2, 2048, 1024, 16
DK = D // H          # 64
NCORES = 8
HPC = H // (NCORES // B)   # heads per core = 4
DSH = HPC * DK             # 256: per-core shard of the model dim
NKC = S // 128             # 16 key chunks of 128
TRI_W = 384 + 512          # causal strip width
VW = 66                    # vp per-head stride: 64 V cols + ones col + pad

BF = mybir.dt.bfloat16
F32 = mybir.dt.float32
EXP = mybir.ActivationFunctionType.Exp

_NC_CACHE: list = []


def _emit(tc: tile.TileContext, ctx):
    nc = tc.nc

    xT = nc.dram_tensor("xT", [D, S], BF, kind="ExternalInput").ap()
    wqt = nc.dram_tensor("wqt", [D, DSH], BF, kind="ExternalInput").ap()
    wkt = nc.dram_tensor("wkt", [D, DSH], BF, kind="ExternalInput").ap()
    wvt = nc.dram_tensor("wvt", [D, DSH], BF, kind="ExternalInput").ap()
    wot = nc.dram_tensor("wot", [DSH, D], BF, kind="ExternalInput").ap()
    pad0 = nc.dram_tensor("pad0", [128, NKC], F32, kind="ExternalInput").ap()
    tri = nc.dram_tensor("tri", [128, TRI_W], BF, kind="ExternalInput").ap()
    sel = nc.dram_tensor("sel", [16, 16 * 64], F32, kind="ExternalInput").ap()
    yT = nc.dram_tensor("yT", [D, S], BF, kind="ExternalOutput").ap()

    persist = ctx.enter_context(tc.tile_pool(name="persist", bufs=1))
    sc_pool = ctx.enter_context(tc.tile_pool(name="scps", bufs=2, space="PSUM"))
    ct_pool = ctx.enter_context(tc.tile_pool(name="ctps", bufs=2, space="PSUM"))
    pp_pool = ctx.enter_context(tc.tile_pool(name="ppps", bufs=2, space="PSUM"))
    pu_pool = ctx.enter_context(tc.tile_pool(name="pu", bufs=4))
    work = ctx.enter_context(tc.tile_pool(name="work", bufs=4))
    dpool = ctx.enter_context(tc.tile_pool(name="dram", bufs=1, space="DRAM"))

    xs = persist.tile([128, 8, S], BF)
    wq_s = persist.tile([128, 8, DSH], BF)
    wk_s = persist.tile([128, 8, DSH], BF)
    wv_s = persist.tile([128, 8, DSH], BF)
    wo_s = persist.tile([128, 2, D], BF)
    pad_s = persist.tile([128, NKC], F32)
    tri_s = persist.tile([128, TRI_W], BF)
    qt2 = persist.tile([128, 2, S], BF)
    kt2 = persist.tile([128, 2, S], BF)
    vp = persist.tile([128, NKC, VW * HPC], BF)
    ctn = persist.tile([128, 2, S], BF)
    ctu = persist.tile([65, 16, 512], F32)    # unnormalized ctx + L, per (h, qt)
    # L rows: r = hq for pair 0, r = 8 + 2*qt + idx for pair 1. lrec is
    # recomputed 1/max(lall,eps) over all 16 rows at each pipeline stage
    # (idempotent — DVE cost depends only on the 512 free dim).
    lall = persist.tile([16, 512], F32)
    lrec = persist.tile([16, 512], F32)
    sel_s = persist.tile([16, 16 * 64], F32)

    # ---- input DMAs, priority-ordered: the first projection needs wq +
    # x[:, :, 0:512]; x stripe 0 goes per-dc-chunk so matmuls start on the
    # first 128 KB. Queues: sync = critical x path, gpsimd = wq/wk,
    # scalar = the rest.
    xr = xT.rearrange("(c p) s -> p c s", p=128)
    wqr = wqt.rearrange("(c p) j -> p c j", p=128)
    wkr = wkt.rearrange("(c p) j -> p c j", p=128)
    wvr = wvt.rearrange("(c p) j -> p c j", p=128)
    wor = wot.rearrange("(c p) o -> p c o", p=128)
    nc.sync.dma_start(out=pad_s, in_=pad0)
    for c in range(8):
        nc.sync.dma_start(out=xs[:, c, 0:512], in_=xr[:, c, 0:512])
    nc.sync.dma_start(out=xs[:, :, 512:1024], in_=xr[:, :, 512:1024])
    nc.gpsimd.dma_start(out=wq_s, in_=wqr)
    nc.gpsimd.dma_start(out=wk_s, in_=wkr)
    nc.gpsimd.dma_start(out=sel_s, in_=sel)
    nc.scalar.dma_start(out=wv_s, in_=wvr)
    nc.scalar.dma_start(out=tri_s, in_=tri)
    nc.scalar.dma_start(out=xs[:, :, 1024:1536], in_=xr[:, :, 1024:1536])
    nc.scalar.dma_start(out=xs[:, :, 1536:2048], in_=xr[:, :, 1536:2048])
    nc.scalar.dma_start(out=wo_s, in_=wor)
    nc.vector.memset(vp, 1.0)
    nc.vector.memset(lall, 1.0)   # unwritten rows must stay finite

    yeng = [nc.sync, nc.gpsimd, nc.scalar]
    yec = [0]

    def qku(dht, wi, st, on_act=False):
        """Project one of Q/K (wi=0/1) for head pair dht, s-tile st."""
        def th():
            wsb, dst = ((wq_s, qt2), (wk_s, kt2))[wi]
            ps = pp_pool.tile([128, 512], F32, tag="pp")
            for dc in range(8):
                nc.tensor.matmul(
                    ps,
                    wsb[:, dc, 128 * dht : 128 * dht + 128],
                    xs[:, dc, 512 * st : 512 * st + 512],
                    start=(dc == 0),
                    stop=(dc == 7),
                )
            sl = dst[:, dht, 512 * st : 512 * st + 512]
            if on_act:
                nc.scalar.copy(sl, ps)
            else:
                nc.vector.tensor_copy(out=sl, in_=ps)
        return th

    def vu(sc):
        """Project V for one 128-row key chunk sc (no masking needed)."""
        def th():
            ps = pp_pool.tile([128, DSH], F32, tag="pp")
            for dc in range(8):
                nc.tensor.matmul(
                    ps,
                    xs[:, dc, 128 * sc : 128 * sc + 128],
                    wv_s[:, dc, :],
                    start=(dc == 0),
                    stop=(dc == 7),
                )
            nc.vector.tensor_copy(
                out=vp[:, sc, :].rearrange("p (h u) -> p h u", u=VW)[:, :, 0:64],
                in_=ps.rearrange("p (h u) -> p h u", u=64),
            )
        return th

    def opu(qt, ot):
        """One 128-row block of the output projection for query tile qt."""
        def th():
            ps = pp_pool.tile([128, 512], F32, tag="pp")
            for c2 in range(2):
                nc.tensor.matmul(
                    ps,
                    wo_s[:, c2, 128 * ot : 128 * ot + 128],
                    ctn[:, c2, 512 * qt : 512 * qt + 512],
                    start=(c2 == 0),
                    stop=(c2 == 1),
                )
            ystg = work.tile([128, 512], BF, tag="y")
            if ot % 2 == 0:
                nc.scalar.copy(ystg, ps)
            else:
                nc.vector.tensor_copy(out=ystg, in_=ps)
            yr = yT.rearrange("(o p) s -> o p s", p=128)
            yeng[yec[0] % 3].dma_start(
                out=yr[ot, :, 512 * qt : 512 * qt + 512], in_=ystg
            )
            yec[0] += 1
        return th

    def attention(hp, qt, fill=(), late=(), late_lo=None):
        Q0 = 512 * qt
        nkc = 4 * qt + 4
        slots: dict = {}
        fe = list(fill)
        for j, th in enumerate(fe):
            k = 1 + (j * max(0, nkc - 2)) // max(1, len(fe))
            slots.setdefault(min(k, nkc - 1), []).append(th)
        fl = list(late)
        if fl:
            lo = late_lo if late_lo is not None else max(2, nkc // 2)
            for j, th in enumerate(fl):
                k = lo + (j * max(1, nkc - lo)) // len(fl)
                slots.setdefault(min(k, nkc - 1), []).append(th)
        ct_e = ct_pool.tile([65, 512], F32, tag="ct")
        ct_o = ct_pool.tile([65, 512], F32, tag="ct")
        for kc in range(nkc):
            K0 = 128 * kc
            band = K0 >= Q0
            # band tiles only cover their live query range [K0, Q0+512)
            qs = K0 if band else Q0
            w = Q0 + 512 - qs
            co = qs - Q0  # ct column offset
            qe = qt2[0:64, hp, qs : qs + w]
            qo = qt2[64:128, hp, qs : qs + w]
            # heads stay at fixed 512-col offsets (PSUM bank alignment)
            sc = sc_pool.tile([128, 1024], F32, tag="slot")
            nc.tensor.matmul(
                sc[:, 0:w], kt2[0:64, hp, K0 : K0 + 128], qe,
                start=True, stop=True,
            )
            nc.tensor.matmul(
                sc[:, 512 : 512 + w], kt2[64:128, hp, K0 : K0 + 128], qo,
                start=True, stop=True,
            )
            pu = pu_pool.tile([128, 1024], BF, tag="pu")
            sc2 = sc.rearrange("p (t f) -> p t f", t=2)[:, :, 0:w]
            pu2 = pu.rearrange("p (t f) -> p t f", t=2)[:, :, 0:w]
            # padding mask folded in: exp(s/8 - 30) ~ 0 on masked key rows
            nc.scalar.activation(
                out=pu2, in_=sc2, func=EXP, scale=0.125,
                bias=pad_s[:, kc : kc + 1],
            )
            if band:  # causal mask; q starts at K0 so the slice is fixed
                tsl = tri_s[:, 384 : 384 + w]
                tslb = bass.AP(   # broadcast over the 2-head dim (stride 0)
                    tensor=tsl.tensor, offset=tsl.offset,
                    ap=[list(tsl.ap[0]), [0, 2], list(tsl.ap[1])],
                )
                nc.vector.tensor_mul(pu2, pu2, tslb)
            he, ho = 2 * hp, 2 * hp + 1
            nc.tensor.matmul(
                ct_e[:, co : co + w],
                vp[:, kc, VW * he : VW * he + 65], pu[:, 0:w],
                start=(kc == 0), stop=(kc == nkc - 1),
            )
            nc.tensor.matmul(
                ct_o[:, co : co + w],
                vp[:, kc, VW * ho : VW * ho + 65], pu[:, 512 : 512 + w],
                start=(kc == 0), stop=(kc == nkc - 1),
            )
            for th in slots.get(kc, ()):
                th()
        for idx, cta in ((0, ct_e), (1, ct_o)):
            hq = (2 * hp + idx) * 4 + qt
            r = hq if hp == 0 else 8 + 2 * qt + idx
            nc.vector.tensor_copy(out=ctu[:, hq, :], in_=cta)
            eng = nc.sync if hp == 0 else nc.gpsimd
            eng.dma_start(out=lall[r : r + 1, :], in_=ctu[64:65, hq, :])

    def _bcast(r):
        """Replicate 1/L row r of lall across 64 partitions via a one-hot
        selector matmul (no DRAM bounce)."""
        rlb = pp_pool.tile([64, 512], F32, tag="pp")
        nc.tensor.matmul(
            rlb, sel_s[:, 64 * r : 64 * r + 64], lrec, start=True, stop=True
        )
        return rlb

    def norm_apply(hp, qt, idx, rlb):
        Q0 = 512 * qt
        hq = (2 * hp + idx) * 4 + qt
        if idx == 0:
            nc.vector.tensor_mul(
                ctn[0:64, hp, Q0 : Q0 + 512], ctu[0:64, hq, :], rlb
            )
        else:
            stg = work.tile([64, 512], BF, tag="stg")
            nc.vector.tensor_mul(stg, ctu[0:64, hq, :], rlb)
            nc.sync.dma_start(out=ctn[64:128, hp, Q0 : Q0 + 512], in_=stg)

    def recip_all():
        def th():
            nc.vector.tensor_scalar_max(lrec, lall, 1e-30)
            nc.vector.reciprocal_approx_fast(out=lrec, in_=lrec)
        return th

    def na0(qt, idx):
        def th():
            norm_apply(0, qt, idx, _bcast(idx * 4 + qt))
        return th

    def nq1a(qt):
        def th():
            for idx in (0, 1):
                norm_apply(1, qt, idx, _bcast(8 + 2 * qt + idx))
        return th

    # ---- emission: one long software pipeline --------------------------
    qku(0, 0, 0, on_act=True)()
    qku(0, 1, 0, on_act=True)()
    for sc in range(4):
        vu(sc)()
    attention(0, 0, fill=[qku(0, 0, 1), qku(0, 1, 1), vu(4), vu(5)])
    attention(0, 1, fill=[qku(0, 0, 2), qku(0, 1, 2),
                          vu(6), vu(7), vu(8), vu(9)])
    attention(0, 2, fill=[qku(0, 0, 3), qku(0, 1, 3),
                          vu(10), vu(11), vu(12), vu(13), vu(14), vu(15)])
    attention(0, 3, fill=[qku(1, 0, 0), qku(1, 1, 0),
                          qku(1, 0, 1), qku(1, 1, 1)])
    # normalize pair 0 while pair 1's attention runs (selector-MM broadcast)
    attention(1, 0, fill=[recip_all(), qku(1, 0, 2), qku(1, 1, 2),
                          na0(0, 0), na0(0, 1), na0(1, 0), na0(1, 1)])
    attention(1, 1, fill=[qku(1, 0, 3), qku(1, 1, 3),
                          na0(2, 0), na0(2, 1), na0(3, 0), na0(3, 1)],
              late=[recip_all(), nq1a(0)], late_lo=5)
    attention(1, 2, late=[recip_all(), nq1a(1)]
                         + [opu(0, ot) for ot in range(8)], late_lo=4)
    attention(1, 3, late=[recip_all(), nq1a(2)]
                         + [opu(1, ot) for ot in range(8)]
                         + [opu(2, ot) for ot in range(8)], late_lo=3)
    recip_all()()
    nq1a(3)()
    for ot in range(8):
        opu(3, ot)()


def build_nc():
    nc = bacc.Bacc(
        "TRN2",
        target_bir_lowering=False,
        debug=False,
        enable_asserts=False,
        num_devices=NCORES,
    )
    from contextlib import ExitStack

    with tile.TileContext(nc) as tc:
        with ExitStack() as ctx:
            _emit(tc, ctx)
    nc.compile()
    return nc


def _get_nc():
    if not _NC_CACHE:
        _NC_CACHE.append(build_nc())
    return _NC_CACHE[0]


def make_tri() -> np.ndarray:
    p = np.arange(128)[:, None]
    v = np.arange(TRI_W)[None, :]
    return (p <= v - 384).astype(np.float32).astype(ml_dtypes.bfloat16)


def make_in_maps(x, mask, WQ, WK, WV, WO):
    bf = ml_dtypes.bfloat16
    tri = make_tri()
    in_maps = []
    for c in range(NCORES):
        b, g = c // (NCORES // B), c % (NCORES // B)
        sl = slice(DSH * g, DSH * g + DSH)
        in_maps.append(
            {
                "xT": np.ascontiguousarray(x[b].T).astype(bf),
                "wqt": np.ascontiguousarray(WQ[sl, :].T).astype(bf),
                "wkt": np.ascontiguousarray(WK[sl, :].T).astype(bf),
                "wvt": np.ascontiguousarray(WV[sl, :].T).astype(bf),
                "wot": np.ascontiguousarray(WO[:, sl].T).astype(bf),
                # exp bias per key row: -30 on masked keys -> exp ~ 0
                "pad0": np.ascontiguousarray(
                    (-30.0 * (mask[b] != 0)).astype(np.float32).reshape(NKC, 128).T
                ),
                "tri": tri,
                "sel": np.kron(np.eye(16, dtype=np.float32), np.ones((1, 64), np.float32)),
            }
        )
    return in_maps


def assemble(results, x, mask, WV, WO, bO) -> np.ndarray:
    y = np.zeros((B, S, D), np.float32)
    for c in range(NCORES):
        y[c // (NCORES // B)] += results[c]["yT"].T
    y += bO[None, None, :]
    # Rows i < first-unmasked-index are fully masked in the reference; its
    # softmax then degenerates to uniform attention over all positions.
    for b in range(B):
        nz = np.nonzero(mask[b] == 0)[0]
        t = int(nz[0]) if nz.size else S
        if t > 0:
            vbar = x[b].mean(axis=0) @ WV.T
            yfix = vbar @ WO.T + bO
            y[b, :t, :] = yfix
    return y


def kernel(x, mask, WQ, WK, WV, WO, bO) -> np.ndarray:
    x = np.asarray(x, np.float32)
    mask = np.asarray(mask, np.int32)
    WQ = np.asarray(WQ, np.float32)
    WK = np.asarray(WK, np.float32)
    WV = np.asarray(WV, np.float32)
    WO = np.asarray(WO, np.float32)
    bO = np.asarray(bO, np.float32)

    nc = _get_nc()
    in_maps = make_in_maps(x, mask, WQ, WK, WV, WO)
    res = run_bass_kernel_spmd(nc, in_maps, list(range(NCORES)))
    return assemble(res.results, x, mask, WV, WO, bO)


# revision 13
# speedup vs baseline: 1.0650x; 1.0650x over previous
"""Multi-head attention (B=2, S=2048, D=1024, H=16) on 8 NeuronCores.

Sharding: core c -> (batch b = c // 4, head-group g = c % 4, 4 heads each).
Each core computes its 4 heads' attention for its batch plus the partial
output projection (ctx_shard @ WO_shard.T).T; the host sums the 4 partials
per batch, adds the bias, and patches fully-masked query rows (where the
reference's softmax degenerates to uniform attention).

Device kernel layout notes:
  - x and the weight shards are pre-transposed on the host and fed as bf16.
  - Q,K are produced in [dk, s] layout (head-pair stacked on partitions) so
    scores come out transposed: S_t[k, q]. The two heads of a pair run as
    concurrent row-group matmuls (K=64 each).
  - Padding mask is folded into the exp as a per-partition (=per-key)
    activation bias of -30 on masked keys: exp(s/8 - 30) ~ 0, so P rows of
    masked keys vanish and V needs no masking. Causal mask is a 0/1
    triangle multiply on the band tiles (DVE, one 3D-AP op per tile).
  - Softmax normalization is deferred: V carries an extra ones column so
    P@V also accumulates row sums L[q]; unnormalized ctx and L stage to
    SBUF, then reciprocal_approx_fast + DRAM-bounce partition-broadcast
    divides at the end (pair 0 pipelined under pair-1 attention; pair 1
    per-qt, each consumed by output-projection blocks with >=1 query-tile
    of lag).
  - Emission is one software pipeline: inputs stream in priority order
    (wq/x interleaved per contraction chunk so the first matmul starts on
    the first 192 KB), all projections and the output projection are
    emitted as small filler units inside the exp-paced attention loops,
    and y tiles DMA out on rotating queues as soon as they finish.
"""

import os
import sys

import numpy as np

sys.path.insert(0, "/opt/trn_rl_repo")
os.environ.setdefault("MYCRO_LOCAL_CACHE", "1")

import ml_dtypes

import concourse.bass as bass
import concourse.tile as tile
from concourse import bacc, mybir
from concourse.bass_utils import run_bass_kernel_spmd

B, S, D, H = 2, 2048, 1024, 16
DK = D // H          # 64
NCORES = 8
HPC = H // (NCORES // B)   # heads per core = 4
DSH = HPC * DK             # 256: per-core shard of the model dim
NKC = S // 128             # 16 key chunks of 128
TRI_W = 384 + 512          # causal strip width
VW = 66                    # vp per-head stride: 64 V + ones col + pad col

BF = mybir.dt.bfloat16
F32 = mybir.dt.float32
EXP = mybir.ActivationFunctionType.Exp

_NC_CACHE: list = []


def _emit(tc: tile.TileContext, ctx):
    nc = tc.nc

    xT = nc.dram_tensor("xT", [D, S], BF, kind="ExternalInput").ap()
    wqt = nc.dram_tensor("wqt", [D, DSH], BF, kind="ExternalInput").ap()
    wkt = nc.dram_tensor("wkt", [D, DSH], BF, kind="ExternalInput").ap()
    wvt = nc.dram_tensor("wvt", [D, DSH], BF, kind="ExternalInput").ap()
    wot = nc.dram_tensor("wot", [DSH, D], BF, kind="ExternalInput").ap()
    pad0 = nc.dram_tensor("pad0", [128, NKC], F32, kind="ExternalInput").ap()
    tri = nc.dram_tensor("tri", [128, TRI_W], BF, kind="ExternalInput").ap()
    yT = nc.dram_tensor("yT", [D, S], BF, kind="ExternalOutput").ap()

    persist = ctx.enter_context(tc.tile_pool(name="persist", bufs=1))
    sc_pool = ctx.enter_context(tc.tile_pool(name="scps", bufs=2, space="PSUM"))
    ct_pool = ctx.enter_context(tc.tile_pool(name="ctps", bufs=2, space="PSUM"))
    pp_pool = ctx.enter_context(tc.tile_pool(name="ppps", bufs=2, space="PSUM"))
    pu_pool = ctx.enter_context(tc.tile_pool(name="pu", bufs=4))
    work = ctx.enter_context(tc.tile_pool(name="work", bufs=4))
    dpool = ctx.enter_context(tc.tile_pool(name="dram", bufs=1, space="DRAM"))

    xs = persist.tile([128, 8, S], BF)
    wq_s = persist.tile([128, 8, DSH], BF)
    wk_s = persist.tile([128, 8, DSH], BF)
    wv_s = persist.tile([128, 8, DSH], BF)
    wo_s = persist.tile([128, 2, D], BF)
    pad_s = persist.tile([128, NKC], F32)
    tri_s = persist.tile([128, TRI_W], BF)
    qt2 = persist.tile([128, 2, S], BF)
    kt2 = persist.tile([128, 2, S], BF)
    vp = persist.tile([128, NKC, VW * HPC], BF)
    ctn = persist.tile([128, 2, S], BF)
    ctu = persist.tile([65, 16, 512], F32)    # unnormalized ctx + L, per (h, qt)
    lall0 = persist.tile([8, 512], F32)
    lq1 = [
        persist.tile([2, 512], F32, name=f"lq1_{i}", tag=f"lq1_{i}")
        for i in range(4)
    ]
    ldram = dpool.tile([8, 512], F32)
    ldram1 = dpool.tile([8, 512], F32)

    # ---- input DMAs, priority-ordered. The first projection consumes
    # (wq chunk c, x chunk c) pairs in order, so interleave them on one
    # FIFO queue; everything else rides the other queues by deadline.
    xr = xT.rearrange("(c p) s -> p c s", p=128)
    wqr = wqt.rearrange("(c p) j -> p c j", p=128)
    wkr = wkt.rearrange("(c p) j -> p c j", p=128)
    wvr = wvt.rearrange("(c p) j -> p c j", p=128)
    wor = wot.rearrange("(c p) o -> p c o", p=128)
    nc.sync.dma_start(out=pad_s, in_=pad0)
    for c in range(8):
        nc.sync.dma_start(out=wq_s[:, c, :], in_=wqr[:, c, :])
        nc.sync.dma_start(out=xs[:, c, 0:512], in_=xr[:, c, 0:512])
    nc.sync.dma_start(out=xs[:, :, 512:1024], in_=xr[:, :, 512:1024])
    nc.gpsimd.dma_start(out=wk_s, in_=wkr)
    nc.scalar.dma_start(out=wv_s, in_=wvr)
    nc.scalar.dma_start(out=tri_s, in_=tri)
    nc.scalar.dma_start(out=xs[:, :, 1024:1536], in_=xr[:, :, 1024:1536])
    nc.scalar.dma_start(out=xs[:, :, 1536:2048], in_=xr[:, :, 1536:2048])
    nc.scalar.dma_start(out=wo_s, in_=wor)
    nc.vector.memset(vp, 1.0)

    yeng = [nc.sync, nc.gpsimd, nc.scalar]
    yec = [0]

    def qk2u(dht, st):
        """Fused Q+K projection for s-tile st: consumes each x chunk twice
        back-to-back (DMA-paced startup unit)."""
        def th():
            ps_q = pp_pool.tile([128, 512], F32, tag="pp")
            ps_k = pp_pool.tile([128, 512], F32, tag="pp")
            for dc in range(8):
                xsl = xs[:, dc, 512 * st : 512 * st + 512]
                nc.tensor.matmul(
                    ps_q, wq_s[:, dc, 128 * dht : 128 * dht + 128], xsl,
                    start=(dc == 0), stop=(dc == 7),
                )
                nc.tensor.matmul(
                    ps_k, wk_s[:, dc, 128 * dht : 128 * dht + 128], xsl,
                    start=(dc == 0), stop=(dc == 7),
                )
            nc.scalar.copy(qt2[:, dht, 512 * st : 512 * st + 512], ps_q)
            nc.vector.tensor_copy(
                out=kt2[:, dht, 512 * st : 512 * st + 512], in_=ps_k
            )
        return th

    def qku(dht, wi, st):
        """Project one of Q/K (wi=0/1) for head pair dht, s-tile st."""
        def th():
            wsb, dst = ((wq_s, qt2), (wk_s, kt2))[wi]
            ps = pp_pool.tile([128, 512], F32, tag="pp")
            for dc in range(8):
                nc.tensor.matmul(
                    ps,
                    wsb[:, dc, 128 * dht : 128 * dht + 128],
                    xs[:, dc, 512 * st : 512 * st + 512],
                    start=(dc == 0),
                    stop=(dc == 7),
                )
            sl = dst[:, dht, 512 * st : 512 * st + 512]
            if wi == 0:
                nc.scalar.copy(sl, ps)
            else:
                nc.vector.tensor_copy(out=sl, in_=ps)
        return th

    def vu(sc):
        """Project V for one 128-row key chunk sc (no masking needed)."""
        def th():
            ps = pp_pool.tile([128, DSH], F32, tag="pp")
            for dc in range(8):
                nc.tensor.matmul(
                    ps,
                    xs[:, dc, 128 * sc : 128 * sc + 128],
                    wv_s[:, dc, :],
                    start=(dc == 0),
                    stop=(dc == 7),
                )
            nc.vector.tensor_copy(
                out=vp[:, sc, :].rearrange("p (h u) -> p h u", u=VW)[:, :, 0:64],
                in_=ps.rearrange("p (h u) -> p h u", u=64),
            )
        return th

    def opu(qt, ot):
        """One 128-row block of the output projection for query tile qt."""
        def th():
            ps = pp_pool.tile([128, 512], F32, tag="pp")
            for c2 in range(2):
                nc.tensor.matmul(
                    ps,
                    wo_s[:, c2, 128 * ot : 128 * ot + 128],
                    ctn[:, c2, 512 * qt : 512 * qt + 512],
                    start=(c2 == 0),
                    stop=(c2 == 1),
                )
            ystg = work.tile([128, 512], BF, tag="y")
            if ot % 2 == 0:
                nc.scalar.copy(ystg, ps)
            else:
                nc.vector.tensor_copy(out=ystg, in_=ps)
            yr = yT.rearrange("(o p) s -> o p s", p=128)
            yeng[yec[0] % 3].dma_start(
                out=yr[ot, :, 512 * qt : 512 * qt + 512], in_=ystg
            )
            yec[0] += 1
        return th

    def attention(hp, qt, fill=(), late=(), late_lo=None):
        Q0 = 512 * qt
        nkc = 4 * qt + 4
        slots: dict = {}
        fe = list(fill)
        for j, th in enumerate(fe):
            k = 1 + (j * max(0, nkc - 2)) // max(1, len(fe))
            slots.setdefault(min(k, nkc - 1), []).append(th)
        fl = list(late)
        if fl:
            lo = late_lo if late_lo is not None else max(2, nkc // 2)
            for j, th in enumerate(fl):
                k = lo + (j * max(1, nkc - lo)) // len(fl)
                slots.setdefault(min(k, nkc - 1), []).append(th)
        ct_e = ct_pool.tile([65, 512], F32, tag="ct")
        ct_o = ct_pool.tile([65, 512], F32, tag="ct")
        for kc in range(nkc):
            K0 = 128 * kc
            band = K0 >= Q0
            # band tiles only cover their live query range [K0, Q0+512)
            qs = K0 if band else Q0
            w = Q0 + 512 - qs
            co = qs - Q0  # ct column offset
            qe = qt2[0:64, hp, qs : qs + w]
            qo = qt2[64:128, hp, qs : qs + w]
            # heads stay at fixed 512-col offsets (PSUM bank alignment)
            sc = sc_pool.tile([128, 1024], F32, tag="slot")
            nc.tensor.matmul(
                sc[:, 0:w], kt2[0:64, hp, K0 : K0 + 128], qe,
                start=True, stop=True,
            )
            nc.tensor.matmul(
                sc[:, 512 : 512 + w], kt2[64:128, hp, K0 : K0 + 128], qo,
                start=True, stop=True,
            )
            pu = pu_pool.tile([128, 1024], BF, tag="pu")
            sc2 = sc.rearrange("p (t f) -> p t f", t=2)[:, :, 0:w]
            pu2 = pu.rearrange("p (t f) -> p t f", t=2)[:, :, 0:w]
            # padding mask folded in: exp(s/8 - 30) ~ 0 on masked key rows
            nc.scalar.activation(
                out=pu2, in_=sc2, func=EXP, scale=0.125,
                bias=pad_s[:, kc : kc + 1],
            )
            if band:  # causal mask; q starts at K0 so the slice is fixed
                tsl = tri_s[:, 384 : 384 + w]
                tslb = bass.AP(   # broadcast over the 2-head dim (stride 0)
                    tensor=tsl.tensor, offset=tsl.offset,
                    ap=[list(tsl.ap[0]), [0, 2], list(tsl.ap[1])],
                )
                nc.vector.tensor_mul(pu2, pu2, tslb)
            he, ho = 2 * hp, 2 * hp + 1
            nc.tensor.matmul(
                ct_e[:, co : co + w],
                vp[:, kc, VW * he : VW * he + 65], pu[:, 0:w],
                start=(kc == 0), stop=(kc == nkc - 1),
            )
            nc.tensor.matmul(
                ct_o[:, co : co + w],
                vp[:, kc, VW * ho : VW * ho + 65], pu[:, 512 : 512 + w],
                start=(kc == 0), stop=(kc == nkc - 1),
            )
            for th in slots.get(kc, ()):
                th()
        for idx, cta in ((0, ct_e), (1, ct_o)):
            hq = (2 * hp + idx) * 4 + qt
            nc.vector.tensor_copy(out=ctu[:, hq, :], in_=cta)
            ldst = (
                lall0[idx * 4 + qt : idx * 4 + qt + 1, :]
                if hp == 0
                else lq1[qt][idx : idx + 1, :]
            )
            eng = nc.sync if hp == 0 else nc.gpsimd
            eng.dma_start(out=ldst, in_=ctu[64:65, hq, :])

    def _bcast64(src_row):
        """[1, 512] DRAM row -> [64, 512] tile via partition-broadcast DMA."""
        rlb = work.tile([64, 512], F32, tag="rlb")
        bsrc = bass.AP(
            tensor=src_row.tensor, offset=src_row.offset,
            ap=[[0, 64]] + list(src_row.ap[1:]),
        )
        nc.sync.dma_start(out=rlb, in_=bsrc)
        return rlb

    def norm_recip0():
        nc.vector.tensor_scalar_max(lall0, lall0, 1e-30)
        nc.vector.reciprocal_approx_fast(out=lall0, in_=lall0)
        nc.sync.dma_start(out=ldram, in_=lall0)

    def norm_apply(hp, qt, idx, rlb):
        Q0 = 512 * qt
        hq = (2 * hp + idx) * 4 + qt
        if idx == 0:
            nc.vector.tensor_mul(
                ctn[0:64, hp, Q0 : Q0 + 512], ctu[0:64, hq, :], rlb
            )
        else:
            stg = work.tile([64, 512], BF, tag="stg")
            nc.vector.tensor_mul(stg, ctu[0:64, hq, :], rlb)
            nc.sync.dma_start(out=ctn[64:128, hp, Q0 : Q0 + 512], in_=stg)

    def na0(qt, idx):
        def th():
            hq = idx * 4 + qt
            norm_apply(0, qt, idx, _bcast64(ldram[hq : hq + 1, :]))
        return th

    def norm_qt1(qt):
        """Per-qt normalize for head pair 1 (tail-pipelined)."""
        lq = lq1[qt]
        nc.vector.tensor_scalar_max(lq, lq, 1e-30)
        nc.vector.reciprocal_approx_fast(out=lq, in_=lq)
        nc.gpsimd.dma_start(out=ldram1[2 * qt : 2 * qt + 2, :], in_=lq)
        for idx in (0, 1):
            norm_apply(
                1, qt, idx, _bcast64(ldram1[2 * qt + idx : 2 * qt + idx + 1, :])
            )

    # ---- emission: one long software pipeline --------------------------
    qk2u(0, 0)()
    for sc in range(4):
        vu(sc)()
    attention(0, 0, fill=[qku(0, 0, 1), qku(0, 1, 1), vu(4), vu(5)])
    attention(0, 1, fill=[qku(0, 0, 2), qku(0, 1, 2),
                          vu(6), vu(7), vu(8), vu(9)])
    attention(0, 2, fill=[qku(0, 0, 3), qku(0, 1, 3),
                          vu(10), vu(11), vu(12), vu(13), vu(14), vu(15)])
    attention(0, 3, fill=[qku(1, 0, 0), qku(1, 1, 0),
                          qku(1, 0, 1), qku(1, 1, 1)])
    # normalize pair 0 while pair 1's attention runs (DRAM-bounce broadcast)
    norm_recip0()
    attention(1, 0, fill=[qku(1, 0, 2), qku(1, 1, 2),
                          na0(0, 0), na0(0, 1), na0(1, 0), na0(1, 1)])
    norm_qt1(0)
    attention(1, 1, fill=[qku(1, 0, 3), qku(1, 1, 3),
                          na0(2, 0), na0(2, 1), na0(3, 0), na0(3, 1)])
    norm_qt1(1)
    attention(1, 2, late=[opu(0, ot) for ot in range(8)], late_lo=6)
    norm_qt1(2)
    attention(1, 3, late=[opu(1, ot) for ot in range(8)]
                         + [opu(2, ot) for ot in range(8)], late_lo=4)
    norm_qt1(3)
    for ot in range(8):
        opu(3, ot)()


def build_nc():
    nc = bacc.Bacc(
        "TRN2",
        target_bir_lowering=False,
        debug=False,
        enable_asserts=False,
        num_devices=NCORES,
    )
    from contextlib import ExitStack

    with tile.TileContext(nc) as tc:
        with ExitStack() as ctx:
            _emit(tc, ctx)
    nc.compile()
    return nc


def _get_nc():
    if not _NC_CACHE:
        _NC_CACHE.append(build_nc())
    return _NC_CACHE[0]


def make_tri() -> np.ndarray:
    p = np.arange(128)[:, None]
    v = np.arange(TRI_W)[None, :]
    return (p <= v - 384).astype(np.float32).astype(ml_dtypes.bfloat16)


def make_in_maps(x, mask, WQ, WK, WV, WO):
    bf = ml_dtypes.bfloat16
    tri = make_tri()
    in_maps = []
    for c in range(NCORES):
        b, g = c // (NCORES // B), c % (NCORES // B)
        sl = slice(DSH * g, DSH * g + DSH)
        in_maps.append(
            {
                "xT": np.ascontiguousarray(x[b].T).astype(bf),
                "wqt": np.ascontiguousarray(WQ[sl, :].T).astype(bf),
                "wkt": np.ascontiguousarray(WK[sl, :].T).astype(bf),
                "wvt": np.ascontiguousarray(WV[sl, :].T).astype(bf),
                "wot": np.ascontiguousarray(WO[:, sl].T).astype(bf),
                # exp bias per key row: -30 on masked keys -> exp ~ 0
                "pad0": np.ascontiguousarray(
                    (-30.0 * (mask[b] != 0)).astype(np.float32).reshape(NKC, 128).T
                ),
                "tri": tri,
            }
        )
    return in_maps


def assemble(results, x, mask, WV, WO, bO) -> np.ndarray:
    y = np.zeros((B, S, D), np.float32)
    for c in range(NCORES):
        y[c // (NCORES // B)] += results[c]["yT"].T
    y += bO[None, None, :]
    # Rows i < first-unmasked-index are fully masked in the reference; its
    # softmax then degenerates to uniform attention over all positions.
    for b in range(B):
        nz = np.nonzero(mask[b] == 0)[0]
        t = int(nz[0]) if nz.size else S
        if t > 0:
            vbar = x[b].mean(axis=0) @ WV.T
            yfix = vbar @ WO.T + bO
            y[b, :t, :] = yfix
    return y


def kernel(x, mask, WQ, WK, WV, WO, bO) -> np.ndarray:
    x = np.asarray(x, np.float32)
    mask = np.asarray(mask, np.int32)
    WQ = np.asarray(WQ, np.float32)
    WK = np.asarray(WK, np.float32)
    WV = np.asarray(WV, np.float32)
    WO = np.asarray(WO, np.float32)
    bO = np.asarray(bO, np.float32)

    nc = _get_nc()
    in_maps = make_in_maps(x, mask, WQ, WK, WV, WO)
    res = run_bass_kernel_spmd(nc, in_maps, list(range(NCORES)))
    return assemble(res.results, x, mask, WV, WO, bO)


# revision 18
# speedup vs baseline: 1.0827x; 1.0166x over previous
"""Multi-head attention (B=2, S=2048, D=1024, H=16) on 8 NeuronCores.

Sharding: core c -> (batch b = c // 4, head-group g = c % 4, 4 heads each).
Each core computes its 4 heads' attention for its batch plus the partial
output projection (ctx_shard @ WO_shard.T).T; the host sums the 4 partials
per batch, adds the bias, and patches fully-masked query rows (where the
reference's softmax degenerates to uniform attention).

Device kernel layout notes:
  - x and the weight shards are pre-transposed on the host and fed as bf16.
  - Q,K are produced in [dk, s] layout (head-pair stacked on partitions) so
    scores come out transposed: S_t[k, q]. The two heads of a pair run as
    concurrent row-group matmuls (K=64 each).
  - Padding mask is folded into the exp as a per-partition (=per-key)
    activation bias of -30 on masked keys: exp(s/8 - 30) ~ 0, so P rows of
    masked keys vanish and V needs no masking. Causal mask is a 0/1
    triangle multiply on the band tiles (DVE, one 3D-AP op per tile).
  - Softmax normalization is deferred: V carries an extra ones column so
    P@V also accumulates row sums L[q]; unnormalized ctx and L stage to
    SBUF, then reciprocal_approx_fast + DRAM-bounce partition-broadcast
    divides at the end (pair 0 pipelined under pair-1 attention; pair 1
    per-qt, each consumed by output-projection blocks with >=1 query-tile
    of lag).
  - Emission is one software pipeline: inputs stream in priority order
    (wq/x interleaved per contraction chunk so the first matmul starts on
    the first 192 KB), all projections and the output projection are
    emitted as small filler units inside the exp-paced attention loops,
    and y tiles DMA out on rotating queues as soon as they finish.
"""

import os
import sys

import numpy as np

sys.path.insert(0, "/opt/trn_rl_repo")
os.environ.setdefault("MYCRO_LOCAL_CACHE", "1")

import ml_dtypes

import concourse.bass as bass
import concourse.tile as tile
from concourse import bacc, mybir
from concourse.bass_utils import run_bass_kernel_spmd

B, S, D, H = 2, 2048, 1024, 16
DK = D // H          # 64
NCORES = 8
HPC = H // (NCORES // B)   # heads per core = 4
DSH = HPC * DK             # 256: per-core shard of the model dim
NKC = S // 128             # 16 key chunks of 128
TRI_W = 384 + 512          # causal strip width
VW = 66                    # vp per-head stride: 64 V + ones col + pad col

BF = mybir.dt.bfloat16
F32 = mybir.dt.float32
EXP = mybir.ActivationFunctionType.Exp

_NC_CACHE: list = []


def _emit(tc: tile.TileContext, ctx):
    nc = tc.nc

    xT = nc.dram_tensor("xT", [D, S], BF, kind="ExternalInput").ap()
    wqt = nc.dram_tensor("wqt", [D, DSH], BF, kind="ExternalInput").ap()
    wkt = nc.dram_tensor("wkt", [D, DSH], BF, kind="ExternalInput").ap()
    wvt = nc.dram_tensor("wvt", [D, DSH], BF, kind="ExternalInput").ap()
    wot = nc.dram_tensor("wot", [DSH, D], BF, kind="ExternalInput").ap()
    pad0 = nc.dram_tensor("pad0", [128, NKC], F32, kind="ExternalInput").ap()
    tri = nc.dram_tensor("tri", [128, TRI_W], BF, kind="ExternalInput").ap()
    yT = nc.dram_tensor("yT", [D, S], BF, kind="ExternalOutput").ap()

    persist = ctx.enter_context(tc.tile_pool(name="persist", bufs=1))
    sc_pool = ctx.enter_context(tc.tile_pool(name="scps", bufs=2, space="PSUM"))
    ct_pool = ctx.enter_context(tc.tile_pool(name="ctps", bufs=2, space="PSUM"))
    pp_pool = ctx.enter_context(tc.tile_pool(name="ppps", bufs=2, space="PSUM"))
    pu_pool = ctx.enter_context(tc.tile_pool(name="pu", bufs=4))
    work = ctx.enter_context(tc.tile_pool(name="work", bufs=4))
    dpool = ctx.enter_context(tc.tile_pool(name="dram", bufs=1, space="DRAM"))

    xs = persist.tile([128, 8, S], BF)
    wq_s = persist.tile([128, 8, DSH], BF)
    wk_s = persist.tile([128, 8, DSH], BF)
    wv_s = persist.tile([128, 8, DSH], BF)
    wo_s = persist.tile([128, 2, D], BF)
    pad_s = persist.tile([128, NKC], F32)
    tri_s = persist.tile([128, TRI_W], BF)
    qt2 = persist.tile([128, 2, S], BF)
    kt2 = persist.tile([128, 2, S], BF)
    vp = persist.tile([128, NKC, VW * HPC], BF)
    ctn = persist.tile([128, 2, S], BF)
    ctu = persist.tile([65, 16, 512], F32)    # unnormalized ctx + L, per (h, qt)
    lall0 = persist.tile([8, 512], F32)
    ldram = dpool.tile([8, 512], F32)
    ldram1 = dpool.tile([8, 512], F32)

    # ---- input DMAs, priority-ordered. The first projection consumes
    # (wq chunk c, x chunk c) pairs in order, so interleave them on one
    # FIFO queue; everything else rides the other queues by deadline.
    xr = xT.rearrange("(c p) s -> p c s", p=128)
    wqr = wqt.rearrange("(c p) j -> p c j", p=128)
    wkr = wkt.rearrange("(c p) j -> p c j", p=128)
    wvr = wvt.rearrange("(c p) j -> p c j", p=128)
    wor = wot.rearrange("(c p) o -> p c o", p=128)
    nc.sync.dma_start(out=pad_s, in_=pad0)
    for c in range(8):
        nc.sync.dma_start(out=wq_s[:, c, :], in_=wqr[:, c, :])
        nc.sync.dma_start(out=xs[:, c, 0:512], in_=xr[:, c, 0:512])
    nc.sync.dma_start(out=xs[:, :, 512:1024], in_=xr[:, :, 512:1024])
    nc.sync.dma_start(out=xs[:, :, 1024:1536], in_=xr[:, :, 1024:1536])
    nc.sync.dma_start(out=xs[:, :, 1536:2048], in_=xr[:, :, 1536:2048])
    nc.gpsimd.dma_start(out=wk_s, in_=wkr)
    nc.scalar.dma_start(out=wv_s, in_=wvr)
    nc.scalar.dma_start(out=tri_s, in_=tri)
    nc.scalar.dma_start(out=wo_s, in_=wor)
    nc.vector.memset(vp, 1.0)

    yeng = [nc.sync, nc.gpsimd, nc.scalar]
    yec = [0]

    def qk2u(dht, st):
        """Fused Q+K projection for s-tile st: consumes each x chunk twice
        back-to-back (DMA-paced startup unit)."""
        def th():
            ps_q = pp_pool.tile([128, 512], F32, tag="pp")
            ps_k = pp_pool.tile([128, 512], F32, tag="pp")
            for dc in range(8):
                xsl = xs[:, dc, 512 * st : 512 * st + 512]
                nc.tensor.matmul(
                    ps_q, wq_s[:, dc, 128 * dht : 128 * dht + 128], xsl,
                    start=(dc == 0), stop=(dc == 7),
                )
                nc.tensor.matmul(
                    ps_k, wk_s[:, dc, 128 * dht : 128 * dht + 128], xsl,
                    start=(dc == 0), stop=(dc == 7),
                )
            nc.scalar.copy(qt2[:, dht, 512 * st : 512 * st + 512], ps_q)
            nc.vector.tensor_copy(
                out=kt2[:, dht, 512 * st : 512 * st + 512], in_=ps_k
            )
        return th

    def qku(dht, wi, st):
        """Project one of Q/K (wi=0/1) for head pair dht, s-tile st."""
        def th():
            wsb, dst = ((wq_s, qt2), (wk_s, kt2))[wi]
            ps = pp_pool.tile([128, 512], F32, tag="pp")
            for dc in range(8):
                nc.tensor.matmul(
                    ps,
                    wsb[:, dc, 128 * dht : 128 * dht + 128],
                    xs[:, dc, 512 * st : 512 * st + 512],
                    start=(dc == 0),
                    stop=(dc == 7),
                )
            sl = dst[:, dht, 512 * st : 512 * st + 512]
            if wi == 0:
                nc.scalar.copy(sl, ps)
            else:
                nc.vector.tensor_copy(out=sl, in_=ps)
        return th

    def vu(sc):
        """Project V for one 128-row key chunk sc (no masking needed)."""
        def th():
            ps = pp_pool.tile([128, DSH], F32, tag="pp")
            for dc in range(8):
                nc.tensor.matmul(
                    ps,
                    xs[:, dc, 128 * sc : 128 * sc + 128],
                    wv_s[:, dc, :],
                    start=(dc == 0),
                    stop=(dc == 7),
                )
            nc.vector.tensor_copy(
                out=vp[:, sc, :].rearrange("p (h u) -> p h u", u=VW)[:, :, 0:64],
                in_=ps.rearrange("p (h u) -> p h u", u=64),
            )
        return th

    def opu(qt, ot):
        """One 128-row block of the output projection for query tile qt."""
        def th():
            ps = pp_pool.tile([128, 512], F32, tag="pp")
            for c2 in range(2):
                nc.tensor.matmul(
                    ps,
                    wo_s[:, c2, 128 * ot : 128 * ot + 128],
                    ctn[:, c2, 512 * qt : 512 * qt + 512],
                    start=(c2 == 0),
                    stop=(c2 == 1),
                )
            ystg = work.tile([128, 512], BF, tag="y")
            if ot % 2 == 0:
                nc.scalar.copy(ystg, ps)
            else:
                nc.vector.tensor_copy(out=ystg, in_=ps)
            yr = yT.rearrange("(o p) s -> o p s", p=128)
            yeng[yec[0] % 3].dma_start(
                out=yr[ot, :, 512 * qt : 512 * qt + 512], in_=ystg
            )
            yec[0] += 1
        return th

    def attention(hp, qt, fill=(), late=(), late_lo=None):
        Q0 = 512 * qt
        nkc = 4 * qt + 4
        slots: dict = {}
        fe = list(fill)
        for j, th in enumerate(fe):
            k = 1 + (j * max(0, nkc - 2)) // max(1, len(fe))
            slots.setdefault(min(k, nkc - 1), []).append(th)
        fl = list(late)
        if fl:
            lo = late_lo if late_lo is not None else max(2, nkc // 2)
            for j, th in enumerate(fl):
                k = lo + (j * max(1, nkc - lo)) // len(fl)
                slots.setdefault(min(k, nkc - 1), []).append(th)
        ct_e = ct_pool.tile([65, 512], F32, tag="ct")
        ct_o = ct_pool.tile([65, 512], F32, tag="ct")
        for kc in range(nkc):
            K0 = 128 * kc
            band = K0 >= Q0
            # band tiles only cover their live query range [K0, Q0+512)
            qs = K0 if band else Q0
            w = Q0 + 512 - qs
            co = qs - Q0  # ct column offset
            qe = qt2[0:64, hp, qs : qs + w]
            qo = qt2[64:128, hp, qs : qs + w]
            # heads stay at fixed 512-col offsets (PSUM bank alignment)
            sc = sc_pool.tile([128, 1024], F32, tag="slot")
            nc.tensor.matmul(
                sc[:, 0:w], kt2[0:64, hp, K0 : K0 + 128], qe,
                start=True, stop=True,
            )
            nc.tensor.matmul(
                sc[:, 512 : 512 + w], kt2[64:128, hp, K0 : K0 + 128], qo,
                start=True, stop=True,
            )
            pu = pu_pool.tile([128, 1024], BF, tag="pu")
            sc2 = sc.rearrange("p (t f) -> p t f", t=2)[:, :, 0:w]
            pu2 = pu.rearrange("p (t f) -> p t f", t=2)[:, :, 0:w]
            # padding mask folded in: exp(s/8 - 30) ~ 0 on masked key rows
            nc.scalar.activation(
                out=pu2, in_=sc2, func=EXP, scale=0.125,
                bias=pad_s[:, kc : kc + 1],
            )
            if band:  # causal mask; q starts at K0 so the slice is fixed
                tsl = tri_s[:, 384 : 384 + w]
                tslb = bass.AP(   # broadcast over the 2-head dim (stride 0)
                    tensor=tsl.tensor, offset=tsl.offset,
                    ap=[list(tsl.ap[0]), [0, 2], list(tsl.ap[1])],
                )
                nc.vector.tensor_mul(pu2, pu2, tslb)
            he, ho = 2 * hp, 2 * hp + 1
            nc.tensor.matmul(
                ct_e[:, co : co + w],
                vp[:, kc, VW * he : VW * he + 65], pu[:, 0:w],
                start=(kc == 0), stop=(kc == nkc - 1),
            )
            nc.tensor.matmul(
                ct_o[:, co : co + w],
                vp[:, kc, VW * ho : VW * ho + 65], pu[:, 512 : 512 + w],
                start=(kc == 0), stop=(kc == nkc - 1),
            )
            for th in slots.get(kc, ()):
                th()
        for idx, cta in ((0, ct_e), (1, ct_o)):
            hq = (2 * hp + idx) * 4 + qt
            nc.vector.tensor_copy(out=ctu[:, hq, :], in_=cta)
            if hp == 0:   # gather to SBUF: one batched recip for all 8 rows
                nc.sync.dma_start(
                    out=lall0[idx * 4 + qt : idx * 4 + qt + 1, :],
                    in_=ctu[64:65, hq, :],
                )
            else:   # raw L row straight to DRAM; recip happens post-bcast
                nc.gpsimd.dma_start(
                    out=ldram1[2 * qt + idx : 2 * qt + idx + 1, :],
                    in_=ctu[64:65, hq, :],
                )

    def _bcast64(src_row):
        """[1, 512] DRAM row -> [64, 512] tile via partition-broadcast DMA."""
        rlb = work.tile([64, 512], F32, tag="rlb")
        bsrc = bass.AP(
            tensor=src_row.tensor, offset=src_row.offset,
            ap=[[0, 64]] + list(src_row.ap[1:]),
        )
        nc.sync.dma_start(out=rlb, in_=bsrc)
        return rlb

    def norm_recip0():
        nc.vector.tensor_scalar_max(lall0, lall0, 1e-30)
        nc.vector.reciprocal_approx_fast(out=lall0, in_=lall0)
        nc.sync.dma_start(out=ldram, in_=lall0)

    def norm_apply(hp, qt, idx, rlb):
        Q0 = 512 * qt
        hq = (2 * hp + idx) * 4 + qt
        if idx == 0:
            nc.vector.tensor_mul(
                ctn[0:64, hp, Q0 : Q0 + 512], ctu[0:64, hq, :], rlb
            )
        else:
            stg = work.tile([64, 512], BF, tag="stg")
            nc.vector.tensor_mul(stg, ctu[0:64, hq, :], rlb)
            nc.sync.dma_start(out=ctn[64:128, hp, Q0 : Q0 + 512], in_=stg)

    def na0(qt, idx):
        def th():
            hq = idx * 4 + qt
            norm_apply(0, qt, idx, _bcast64(ldram[hq : hq + 1, :]))
        return th

    def nq1b(qt, idx):
        """Pair-1 normalize for (qt, idx): broadcast the raw L row, then
        max+reciprocal on the broadcast tile (keeps the serial chain to
        ldst -> bcast -> DVE -> mul)."""
        def th():
            rlb = _bcast64(ldram1[2 * qt + idx : 2 * qt + idx + 1, :])
            nc.vector.tensor_scalar_max(rlb, rlb, 1e-30)
            nc.vector.reciprocal_approx_fast(out=rlb, in_=rlb)
            norm_apply(1, qt, idx, rlb)
        return th

    # ---- emission: one long software pipeline --------------------------
    qk2u(0, 0)()
    for sc in range(4):
        vu(sc)()
    attention(0, 0, fill=[qku(0, 0, 1), qku(0, 1, 1), vu(4), vu(5)])
    attention(0, 1, fill=[qku(0, 0, 2), qku(0, 1, 2),
                          vu(6), vu(7), vu(8), vu(9)])
    attention(0, 2, fill=[qku(0, 0, 3), qku(0, 1, 3),
                          vu(10), vu(11), vu(12), vu(13), vu(14), vu(15)])
    attention(0, 3, fill=[qku(1, 0, 0), qku(1, 1, 0),
                          qku(1, 0, 1), qku(1, 1, 1)])
    # normalize pair 0 while pair 1's attention runs (DRAM-bounce broadcast)
    norm_recip0()
    attention(1, 0, fill=[qku(1, 0, 2), qku(1, 1, 2),
                          na0(0, 0), na0(0, 1), na0(1, 0), na0(1, 1)])
    attention(1, 1, fill=[nq1b(0, 0), nq1b(0, 1), qku(1, 0, 3), qku(1, 1, 3),
                          na0(2, 0), na0(2, 1), na0(3, 0), na0(3, 1)])
    attention(1, 2, fill=[nq1b(1, 0), nq1b(1, 1)],
              late=[opu(0, ot) for ot in range(8)], late_lo=6)
    attention(1, 3, fill=[nq1b(2, 0), nq1b(2, 1)],
              late=[opu(1, ot) for ot in range(8)]
                   + [opu(2, ot) for ot in range(8)], late_lo=4)
    nq1b(3, 0)()
    nq1b(3, 1)()
    for ot in range(8):
        opu(3, ot)()


def build_nc():
    nc = bacc.Bacc(
        "TRN2",
        target_bir_lowering=False,
        debug=False,
        enable_asserts=False,
        num_devices=NCORES,
    )
    from contextlib import ExitStack

    with tile.TileContext(nc) as tc:
        with ExitStack() as ctx:
            _emit(tc, ctx)
    nc.compile()
    return nc


def _get_nc():
    if not _NC_CACHE:
        _NC_CACHE.append(build_nc())
    return _NC_CACHE[0]


def make_tri() -> np.ndarray:
    p = np.arange(128)[:, None]
    v = np.arange(TRI_W)[None, :]
    return (p <= v - 384).astype(np.float32).astype(ml_dtypes.bfloat16)


def make_in_maps(x, mask, WQ, WK, WV, WO):
    bf = ml_dtypes.bfloat16
    tri = make_tri()
    in_maps = []
    for c in range(NCORES):
        b, g = c // (NCORES // B), c % (NCORES // B)
        sl = slice(DSH * g, DSH * g + DSH)
        in_maps.append(
            {
                "xT": np.ascontiguousarray(x[b].T).astype(bf),
                "wqt": np.ascontiguousarray(WQ[sl, :].T).astype(bf),
                "wkt": np.ascontiguousarray(WK[sl, :].T).astype(bf),
                "wvt": np.ascontiguousarray(WV[sl, :].T).astype(bf),
                "wot": np.ascontiguousarray(WO[:, sl].T).astype(bf),
                # exp bias per key row: -30 on masked keys -> exp ~ 0
                "pad0": np.ascontiguousarray(
                    (-30.0 * (mask[b] != 0)).astype(np.float32).reshape(NKC, 128).T
                ),
                "tri": tri,
            }
        )
    return in_maps


def assemble(results, x, mask, WV, WO, bO) -> np.ndarray:
    y = np.zeros((B, S, D), np.float32)
    for c in range(NCORES):
        y[c // (NCORES // B)] += results[c]["yT"].T
    y += bO[None, None, :]
    # Rows i < first-unmasked-index are fully masked in the reference; its
    # softmax then degenerates to uniform attention over all positions.
    for b in range(B):
        nz = np.nonzero(mask[b] == 0)[0]
        t = int(nz[0]) if nz.size else S
        if t > 0:
            vbar = x[b].mean(axis=0) @ WV.T
            yfix = vbar @ WO.T + bO
            y[b, :t, :] = yfix
    return y


def kernel(x, mask, WQ, WK, WV, WO, bO) -> np.ndarray:
    x = np.asarray(x, np.float32)
    mask = np.asarray(mask, np.int32)
    WQ = np.asarray(WQ, np.float32)
    WK = np.asarray(WK, np.float32)
    WV = np.asarray(WV, np.float32)
    WO = np.asarray(WO, np.float32)
    bO = np.asarray(bO, np.float32)

    nc = _get_nc()
    in_maps = make_in_maps(x, mask, WQ, WK, WV, WO)
    res = run_bass_kernel_spmd(nc, in_maps, list(range(NCORES)))
    return assemble(res.results, x, mask, WV, WO, bO)


# revision 29
# speedup vs baseline: 1.1194x; 1.0339x over previous
"""Multi-head attention (B=2, S=2048, D=1024, H=16) on 8 NeuronCores.

Sharding: core c -> (batch b = c // 4, head-group g = c % 4, 4 heads each).
Each core computes its 4 heads' attention for its batch plus the partial
output projection (ctx_shard @ WO_shard.T).T; the host sums the 4 partials
per batch, adds the bias, and patches fully-masked query rows (where the
reference's softmax degenerates to uniform attention).

Device kernel layout notes:
  - x and the weight shards are pre-transposed on the host and fed as bf16.
  - Q,K are produced in [dk, s] layout (head-pair stacked on partitions) so
    scores come out transposed: S_t[k, q]. The two heads of a pair run as
    concurrent row-group matmuls (K=64 each).
  - Padding mask is folded into the exp as a per-partition (=per-key)
    activation bias of -30 on masked keys: exp(s/8 - 30) ~ 0, so P rows of
    masked keys vanish and V needs no masking. Causal mask is a 0/1
    triangle multiply on the band tiles (DVE, one 3D-AP op per tile).
  - Softmax normalization is deferred: V carries an extra ones column so
    P@V also accumulates row sums L[q]; unnormalized ctx and L stage to
    SBUF, then reciprocal_approx_fast + DRAM-bounce partition-broadcast
    divides at the end (pair 0 pipelined under pair-1 attention; pair 1
    per-qt, each consumed by output-projection blocks with >=1 query-tile
    of lag).
  - Emission is one software pipeline: inputs stream in priority order
    (wq/x interleaved per contraction chunk so the first matmul starts on
    the first 192 KB), all projections and the output projection are
    emitted as small filler units inside the exp-paced attention loops,
    and y tiles DMA out on rotating queues as soon as they finish.
"""

import os
import sys

import numpy as np

sys.path.insert(0, "/opt/trn_rl_repo")
os.environ.setdefault("MYCRO_LOCAL_CACHE", "1")

import ml_dtypes

import concourse.bass as bass
import concourse.tile as tile
from concourse import bacc, mybir
from concourse.bass_utils import run_bass_kernel_spmd

B, S, D, H = 2, 2048, 1024, 16
DK = D // H          # 64
NCORES = 8
HPC = H // (NCORES // B)   # heads per core = 4
DSH = HPC * DK             # 256: per-core shard of the model dim
NKC = S // 128             # 16 key chunks of 128
TRI_W = 384 + 512          # causal strip width
VW = 66                    # vp per-head stride: 64 V + ones col + pad col

BF = mybir.dt.bfloat16
F32 = mybir.dt.float32
EXP = mybir.ActivationFunctionType.Exp

_NC_CACHE: list = []


def _emit(tc: tile.TileContext, ctx):
    nc = tc.nc

    xT = nc.dram_tensor("xT", [D, S], BF, kind="ExternalInput").ap()
    wqt = nc.dram_tensor("wqt", [D, DSH], BF, kind="ExternalInput").ap()
    wkt = nc.dram_tensor("wkt", [D, DSH], BF, kind="ExternalInput").ap()
    wvt = nc.dram_tensor("wvt", [D, DSH], BF, kind="ExternalInput").ap()
    wot = nc.dram_tensor("wot", [DSH, D], BF, kind="ExternalInput").ap()
    pad0 = nc.dram_tensor("pad0", [128, NKC], F32, kind="ExternalInput").ap()
    tri = nc.dram_tensor("tri", [128, TRI_W], BF, kind="ExternalInput").ap()
    yT = nc.dram_tensor("yT", [D, S], BF, kind="ExternalOutput").ap()

    persist = ctx.enter_context(tc.tile_pool(name="persist", bufs=1))
    sc_pool = ctx.enter_context(tc.tile_pool(name="scps", bufs=2, space="PSUM"))
    ct_pool = ctx.enter_context(tc.tile_pool(name="ctps", bufs=2, space="PSUM"))
    pp_pool = ctx.enter_context(tc.tile_pool(name="ppps", bufs=2, space="PSUM"))
    pu_pool = ctx.enter_context(tc.tile_pool(name="pu", bufs=4))
    work = ctx.enter_context(tc.tile_pool(name="work", bufs=4))
    dpool = ctx.enter_context(tc.tile_pool(name="dram", bufs=1, space="DRAM"))

    xs = persist.tile([128, 8, S], BF)
    wq_s = persist.tile([128, 8, DSH], BF)
    wk_s = persist.tile([128, 8, DSH], BF)
    wv_s = persist.tile([128, 8, DSH], BF)
    wo_s = persist.tile([128, 2, D], BF)
    pad_s = persist.tile([128, NKC], F32)
    tri_s = persist.tile([128, TRI_W], BF)
    qt2 = persist.tile([128, 2, S], BF)
    kt2 = persist.tile([128, 2, S], BF)
    vp = persist.tile([128, NKC, VW * HPC], BF)
    ctn = persist.tile([128, 2, S], BF)
    ctu = persist.tile([65, 16, 512], F32)    # unnormalized ctx + L, per (h, qt)
    lall0 = persist.tile([8, 512], F32)
    rlb0 = persist.tile([64, 8, 512], F32)   # pair-0 1/L broadcasts
    rlb1 = persist.tile([64, 8, 512], F32)   # pair-1 raw-L broadcasts
    ldram = dpool.tile([8, 512], F32)
    ldram1 = dpool.tile([8, 512], F32)

    # ---- input DMAs, priority-ordered. The first projection consumes
    # (wq chunk c, x chunk c) pairs in order, so interleave them on one
    # FIFO queue; everything else rides the other queues by deadline.
    xr = xT.rearrange("(c p) s -> p c s", p=128)
    wqr = wqt.rearrange("(c p) j -> p c j", p=128)
    wkr = wkt.rearrange("(c p) j -> p c j", p=128)
    wvr = wvt.rearrange("(c p) j -> p c j", p=128)
    wor = wot.rearrange("(c p) o -> p c o", p=128)
    nc.sync.dma_start(out=pad_s, in_=pad0)
    for c in range(8):
        nc.sync.dma_start(out=wq_s[:, c, :], in_=wqr[:, c, :])
        nc.sync.dma_start(out=xs[:, c, 0:512], in_=xr[:, c, 0:512])
    nc.sync.dma_start(out=xs[:, :, 512:1024], in_=xr[:, :, 512:1024])
    nc.sync.dma_start(out=xs[:, :, 1024:1536], in_=xr[:, :, 1024:1536])
    nc.sync.dma_start(out=xs[:, :, 1536:2048], in_=xr[:, :, 1536:2048])
    nc.gpsimd.dma_start(out=wk_s, in_=wkr)
    nc.scalar.dma_start(out=wv_s, in_=wvr)
    nc.scalar.dma_start(out=tri_s, in_=tri)
    nc.scalar.dma_start(out=wo_s, in_=wor)
    nc.vector.memset(vp, 1.0)

    def qk2u(dht, st):
        """Fused Q+K projection for s-tile st: consumes each x chunk twice
        back-to-back (DMA-paced startup unit)."""
        def th():
            ps_q = pp_pool.tile([128, 512], F32, tag="pp")
            ps_k = pp_pool.tile([128, 512], F32, tag="pp")
            for dc in range(8):
                xsl = xs[:, dc, 512 * st : 512 * st + 512]
                nc.tensor.matmul(
                    ps_q, wq_s[:, dc, 128 * dht : 128 * dht + 128], xsl,
                    start=(dc == 0), stop=(dc == 7),
                )
                nc.tensor.matmul(
                    ps_k, wk_s[:, dc, 128 * dht : 128 * dht + 128], xsl,
                    start=(dc == 0), stop=(dc == 7),
                )
            nc.scalar.copy(qt2[:, dht, 512 * st : 512 * st + 512], ps_q)
            nc.vector.tensor_copy(
                out=kt2[:, dht, 512 * st : 512 * st + 512], in_=ps_k
            )
        return th

    def qku(dht, wi, st):
        """Project one of Q/K (wi=0/1) for head pair dht, s-tile st."""
        def th():
            wsb, dst = ((wq_s, qt2), (wk_s, kt2))[wi]
            ps = pp_pool.tile([128, 512], F32, tag="pp")
            for dc in range(8):
                nc.tensor.matmul(
                    ps,
                    wsb[:, dc, 128 * dht : 128 * dht + 128],
                    xs[:, dc, 512 * st : 512 * st + 512],
                    start=(dc == 0),
                    stop=(dc == 7),
                )
            sl = dst[:, dht, 512 * st : 512 * st + 512]
            if wi == 0:
                nc.scalar.copy(sl, ps)
            else:
                nc.vector.tensor_copy(out=sl, in_=ps)
        return th

    def vu(sc):
        """Project V for one 128-row key chunk sc (no masking needed)."""
        def th():
            ps = pp_pool.tile([128, DSH], F32, tag="pp")
            for dc in range(8):
                nc.tensor.matmul(
                    ps,
                    xs[:, dc, 128 * sc : 128 * sc + 128],
                    wv_s[:, dc, :],
                    start=(dc == 0),
                    stop=(dc == 7),
                )
            nc.vector.tensor_copy(
                out=vp[:, sc, :].rearrange("p (h u) -> p h u", u=VW)[:, :, 0:64],
                in_=ps.rearrange("p (h u) -> p h u", u=64),
            )
        return th

    def opu(qt, ot):
        """One 128-row block of the output projection for query tile qt."""
        def th():
            ps = pp_pool.tile([128, 512], F32, tag="pp")
            for c2 in range(2):
                nc.tensor.matmul(
                    ps,
                    wo_s[:, c2, 128 * ot : 128 * ot + 128],
                    ctn[:, c2, 512 * qt : 512 * qt + 512],
                    start=(c2 == 0),
                    stop=(c2 == 1),
                )
            ystg = work.tile([128, 512], BF, tag="y")
            if ot % 2 == 0:
                nc.scalar.copy(ystg, ps)
            else:
                nc.vector.tensor_copy(out=ystg, in_=ps)
            yr = yT.rearrange("(o p) s -> o p s", p=128)
            nc.sync.dma_start(
                out=yr[ot, :, 512 * qt : 512 * qt + 512], in_=ystg
            )
        return th

    def attention(hp, qt, fill=(), late=(), late_lo=None):
        Q0 = 512 * qt
        nkc = 4 * qt + 4
        slots: dict = {}
        fe = list(fill)
        for j, th in enumerate(fe):
            k = 1 + (j * max(0, nkc - 2)) // max(1, len(fe))
            slots.setdefault(min(k, nkc - 1), []).append(th)
        fl = list(late)
        if fl:
            lo = late_lo if late_lo is not None else max(2, nkc // 2)
            for j, th in enumerate(fl):
                k = lo + (j * max(1, nkc - lo)) // len(fl)
                slots.setdefault(min(k, nkc - 1), []).append(th)
        ct_e = ct_pool.tile([65, 512], F32, tag="ct")
        ct_o = ct_pool.tile([65, 512], F32, tag="ct")
        for kc in range(nkc):
            K0 = 128 * kc
            band = K0 >= Q0
            # band tiles only cover their live query range [K0, Q0+512)
            qs = K0 if band else Q0
            w = Q0 + 512 - qs
            co = qs - Q0  # ct column offset
            qe = qt2[0:64, hp, qs : qs + w]
            qo = qt2[64:128, hp, qs : qs + w]
            # heads stay at fixed 512-col offsets (PSUM bank alignment)
            sc = sc_pool.tile([128, 1024], F32, tag="slot")
            nc.tensor.matmul(
                sc[:, 0:w], kt2[0:64, hp, K0 : K0 + 128], qe,
                start=True, stop=True,
            )
            nc.tensor.matmul(
                sc[:, 512 : 512 + w], kt2[64:128, hp, K0 : K0 + 128], qo,
                start=True, stop=True,
            )
            pu = pu_pool.tile([128, 1024], BF, tag="pu")
            sc2 = sc.rearrange("p (t f) -> p t f", t=2)[:, :, 0:w]
            pu2 = pu.rearrange("p (t f) -> p t f", t=2)[:, :, 0:w]
            # padding mask folded in: exp(s/8 - 30) ~ 0 on masked key rows
            nc.scalar.activation(
                out=pu2, in_=sc2, func=EXP, scale=0.125,
                bias=pad_s[:, kc : kc + 1],
            )
            if band:  # causal mask; q starts at K0 so the slice is fixed
                tsl = tri_s[:, 384 : 384 + w]
                tslb = bass.AP(   # broadcast over the 2-head dim (stride 0)
                    tensor=tsl.tensor, offset=tsl.offset,
                    ap=[list(tsl.ap[0]), [0, 2], list(tsl.ap[1])],
                )
                nc.vector.tensor_mul(pu2, pu2, tslb)
            he, ho = 2 * hp, 2 * hp + 1
            nc.tensor.matmul(
                ct_e[:, co : co + w],
                vp[:, kc, VW * he : VW * he + 65], pu[:, 0:w],
                start=(kc == 0), stop=(kc == nkc - 1),
            )
            nc.tensor.matmul(
                ct_o[:, co : co + w],
                vp[:, kc, VW * ho : VW * ho + 65], pu[:, 512 : 512 + w],
                start=(kc == 0), stop=(kc == nkc - 1),
            )
            for th in slots.get(kc, ()):
                th()
        for idx, cta in ((0, ct_e), (1, ct_o)):
            hq = (2 * hp + idx) * 4 + qt
            nc.vector.tensor_copy(out=ctu[:, hq, :], in_=cta)
            if hp == 0:   # gather to SBUF: one batched recip for all 8 rows
                nc.sync.dma_start(
                    out=lall0[idx * 4 + qt : idx * 4 + qt + 1, :],
                    in_=ctu[64:65, hq, :],
                )
            else:  # raw L row straight to DRAM; recip happens post-broadcast
                nc.gpsimd.dma_start(
                    out=ldram1[2 * qt + idx : 2 * qt + idx + 1, :],
                    in_=ctu[64:65, hq, :],
                )

    def _bcast64(src_row, dst):
        """[1, 512] DRAM row -> [64, 512] SBUF via partition-broadcast DMA
        (HWDGE/sync queue; issued early, consumed late)."""
        bsrc = bass.AP(
            tensor=src_row.tensor, offset=src_row.offset,
            ap=[[0, 64]] + list(src_row.ap[1:]),
        )
        nc.sync.dma_start(out=dst, in_=bsrc)

    def recip0():
        def th():
            nc.vector.tensor_scalar_max(lall0, lall0, 1e-30)
            nc.vector.reciprocal_approx_fast(out=lall0, in_=lall0)
            nc.sync.dma_start(out=ldram, in_=lall0)
        return th

    def na0bc():
        def th():
            for hq in range(8):
                _bcast64(ldram[hq : hq + 1, :], rlb0[:, hq, :])
        return th

    def nq1bc(qt):
        def th():
            for idx in (0, 1):
                r = 2 * qt + idx
                _bcast64(ldram1[r : r + 1, :], rlb1[:, r, :])
        return th

    def norm_apply(hp, qt, idx, rlb):
        Q0 = 512 * qt
        hq = (2 * hp + idx) * 4 + qt
        if idx == 0:
            nc.vector.tensor_mul(
                ctn[0:64, hp, Q0 : Q0 + 512], ctu[0:64, hq, :], rlb
            )
        else:
            stg = work.tile([64, 512], BF, tag="stg")
            nc.vector.tensor_mul(stg, ctu[0:64, hq, :], rlb)
            nc.gpsimd.dma_start(out=ctn[64:128, hp, Q0 : Q0 + 512], in_=stg)

    def na0m(qt, idx):
        def th():
            norm_apply(0, qt, idx, rlb0[:, idx * 4 + qt, :])
        return th

    def nq1m(qt, idx):
        """Pair-1: the broadcast carried raw L; reciprocal it in place."""
        def th():
            r = 2 * qt + idx
            nc.vector.reciprocal_approx_fast(
                out=rlb1[:, r, :], in_=rlb1[:, r, :]
            )
            norm_apply(1, qt, idx, rlb1[:, r, :])
        return th



    # ---- emission: one long software pipeline --------------------------
    qk2u(0, 0)()
    for sc in range(4):
        vu(sc)()
    attention(0, 0, fill=[qku(0, 0, 1), qku(0, 1, 1), vu(4), vu(5)])
    attention(0, 1, fill=[qku(0, 0, 2), qku(0, 1, 2),
                          vu(6), vu(7), vu(8), vu(9)])
    attention(0, 2, fill=[qku(0, 0, 3), qku(0, 1, 3),
                          vu(10), vu(11), vu(12), vu(13), vu(14), vu(15)])
    attention(0, 3, fill=[qku(1, 0, 0), qku(1, 1, 0),
                          qku(1, 0, 1), qku(1, 1, 1)])
    # normalize pair 0 while pair 1's attention runs (DRAM-bounce broadcast)
    attention(1, 0, fill=[recip0(), na0bc(), qku(1, 0, 2), qku(1, 1, 2)])
    attention(1, 1, fill=[nq1bc(0), qku(1, 0, 3), qku(1, 1, 3),
                          na0m(0, 0), na0m(0, 1), na0m(1, 0), na0m(1, 1),
                          na0m(2, 0), na0m(2, 1), na0m(3, 0), na0m(3, 1)],
              late=[nq1m(0, 0), nq1m(0, 1)], late_lo=5)
    attention(1, 2, fill=[nq1bc(1)],
              late=[nq1m(1, 0), nq1m(1, 1)]
                   + [opu(0, ot) for ot in range(8)], late_lo=4)
    attention(1, 3, fill=[nq1bc(2)],
              late=[nq1m(2, 0), nq1m(2, 1)]
                   + [opu(1, ot) for ot in range(8)]
                   + [opu(2, ot) for ot in range(8)], late_lo=3)
    nq1bc(3)()
    nq1m(3, 0)()
    nq1m(3, 1)()
    for ot in range(8):
        opu(3, ot)()


def build_nc():
    nc = bacc.Bacc(
        "TRN2",
        target_bir_lowering=False,
        debug=False,
        enable_asserts=False,
        num_devices=NCORES,
    )
    from contextlib import ExitStack

    with tile.TileContext(nc) as tc:
        with ExitStack() as ctx:
            _emit(tc, ctx)
    nc.compile()
    return nc


def _get_nc():
    if not _NC_CACHE:
        _NC_CACHE.append(build_nc())
    return _NC_CACHE[0]


def make_tri() -> np.ndarray:
    p = np.arange(128)[:, None]
    v = np.arange(TRI_W)[None, :]
    return (p <= v - 384).astype(np.float32).astype(ml_dtypes.bfloat16)


def make_in_maps(x, mask, WQ, WK, WV, WO):
    bf = ml_dtypes.bfloat16
    tri = make_tri()
    in_maps = []
    for c in range(NCORES):
        b, g = c // (NCORES // B), c % (NCORES // B)
        sl = slice(DSH * g, DSH * g + DSH)
        in_maps.append(
            {
                "xT": np.ascontiguousarray(x[b].T).astype(bf),
                "wqt": np.ascontiguousarray(WQ[sl, :].T).astype(bf),
                "wkt": np.ascontiguousarray(WK[sl, :].T).astype(bf),
                "wvt": np.ascontiguousarray(WV[sl, :].T).astype(bf),
                "wot": np.ascontiguousarray(WO[:, sl].T).astype(bf),
                # exp bias per key row: -30 on masked keys -> exp ~ 0
                "pad0": np.ascontiguousarray(
                    (-30.0 * (mask[b] != 0)).astype(np.float32).reshape(NKC, 128).T
                ),
                "tri": tri,
            }
        )
    return in_maps


def assemble(results, x, mask, WV, WO, bO) -> np.ndarray:
    y = np.zeros((B, S, D), np.float32)
    for c in range(NCORES):
        y[c // (NCORES // B)] += results[c]["yT"].T
    y += bO[None, None, :]
    # Rows i < first-unmasked-index are fully masked in the reference; its
    # softmax then degenerates to uniform attention over all positions.
    for b in range(B):
        nz = np.nonzero(mask[b] == 0)[0]
        t = int(nz[0]) if nz.size else S
        if t > 0:
            vbar = x[b].mean(axis=0) @ WV.T
            yfix = vbar @ WO.T + bO
            y[b, :t, :] = yfix
    return y


def kernel(x, mask, WQ, WK, WV, WO, bO) -> np.ndarray:
    x = np.asarray(x, np.float32)
    mask = np.asarray(mask, np.int32)
    WQ = np.asarray(WQ, np.float32)
    WK = np.asarray(WK, np.float32)
    WV = np.asarray(WV, np.float32)
    WO = np.asarray(WO, np.float32)
    bO = np.asarray(bO, np.float32)

    nc = _get_nc()
    in_maps = make_in_maps(x, mask, WQ, WK, WV, WO)
    res = run_bass_kernel_spmd(nc, in_maps, list(range(NCORES)))
    return assemble(res.results, x, mask, WV, WO, bO)
